# revision 51
# baseline (speedup 1.0000x reference)
"""Single-head attention (Q/K/V proj + softmax + PV) on 8 TRN2 NeuronCores.

Sharding: data-parallel over batch (B=8 -> 1 batch element per core);
Q/K/V weights replicated. Each core computes, for its batch b:
  probsT_b = softmax(x_b Wq+bq @ (x_b Wk+bk)^T / 8, axis=t)^T   [t, s] layout
  outT_b   = (probs_b @ (x_b Wv+bv))^T                          [d, s] layout
The transposed layouts keep every on-chip matmul in the natural
TensorE form (contract dim on partitions) with no large on-chip
transpose of the S x S matrix; the host returns transposed views.

Device-side math (per core):
  xT (PE-transpose of x tiles) -> QT/KT/VT = W^T xT (+bias)
  scoresT[t,s] = K Q^T  (lhsT = KT tile, rhs = QT block)
  ET = exp(0.125 * scoresT)            (ScalarE, fused scale)
  [outT; rowsum] = [V | 1]^T ET        (ones column folded into V)
  probsT = ET * (1/rowsum) broadcast   (PE ones-broadcast + DVE recip)

Matmul operands use float32r (single-pass PE mode, ~1.6e-4 matmul rel
err vs 4-cycle/row full fp32) -- the kernel is memory-bound and fp32
PE would otherwise dominate. Phase 1 computes all K projections first
so phase 2's scores matmuls can overlap the remaining Q/V projections.
"""

import sys
import types

import numpy as np

B, S, H, D = 8, 2048, 768, 64
P = 128          # partitions
SB = 512         # s-block width (PSUM bank)
NSB = S // SB    # 4 s-blocks
NT = S // P      # 16 t-blocks
NH = H // P      # 6 h-blocks
N_CORES = 8

_COMPILED = None


def _ensure_axon_hooks():
    """bass_utils imports antenv.axon_hooks under axon when trace=True;
    provide the real ctypes hook (or a stub) if the image lacks it."""
    if "antenv.axon_hooks" in sys.modules:
        return
    try:
        import antenv.axon_hooks  # noqa: F401
        return
    except ImportError:
        pass
    hook = None
    try:
        from trn_agent_boot.trn_boot import _ntff_profile_via_ctypes
        hook = _ntff_profile_via_ctypes("/opt/axon/libaxon_pjrt.so")
    except Exception:
        hook = None
    mod = types.ModuleType("antenv.axon_hooks")
    mod.get_axon_ntff_profile_hook = lambda: hook
    mod.set_axon_ntff_profile_hook = lambda h: None
    sys.modules["antenv.axon_hooks"] = mod


def _build():
    import concourse.bacc as bacc
    import concourse.tile as tile
    import concourse.mybir as mybir
    from concourse.masks import make_identity

    F32 = mybir.dt.float32
    F32R = mybir.dt.float32r
    EXP = mybir.ActivationFunctionType.Exp
    IDENT_FN = mybir.ActivationFunctionType.Identity
    COPY_FN = mybir.ActivationFunctionType.Copy

    nc = bacc.Bacc(None, target_bir_lowering=False, debug=False)

    x = nc.dram_tensor("x", [S, H], F32, kind="ExternalInput")
    wq = nc.dram_tensor("Wq", [H, D], F32, kind="ExternalInput")
    bq = nc.dram_tensor("bq", [D], F32, kind="ExternalInput")
    wk = nc.dram_tensor("Wk", [H, D], F32, kind="ExternalInput")
    bk = nc.dram_tensor("bk", [D], F32, kind="ExternalInput")
    wv = nc.dram_tensor("Wv", [H, D], F32, kind="ExternalInput")
    bv = nc.dram_tensor("bv", [D], F32, kind="ExternalInput")
    probsT = nc.dram_tensor("probsT", [S, S], F32, kind="ExternalOutput")
    outT = nc.dram_tensor("outT", [D, S], F32, kind="ExternalOutput")

    with tile.TileContext(nc) as tc:
        with (
            tc.tile_pool(name="const", bufs=1) as const,
            tc.tile_pool(name="persist", bufs=1) as persist,
            tc.tile_pool(name="xn", bufs=3) as xn_pool,
            tc.tile_pool(name="xt", bufs=4) as xt_pool,
            tc.tile_pool(name="et", bufs=17) as et_pool,
            tc.tile_pool(name="rb", bufs=2) as rb_pool,
            tc.tile_pool(name="pr", bufs=5) as pr_pool,
            tc.tile_pool(name="ptr", bufs=2, space="PSUM") as ptr_pool,
            tc.tile_pool(name="pmm", bufs=2, space="PSUM") as pmm_pool,
            tc.tile_pool(name="po", bufs=1, space="PSUM") as po_pool,
        ):
            # ---- constants ----
            ident = const.tile([P, P], F32)
            make_identity(nc, ident)
            ones_row = const.tile([1, P], F32)
            nc.vector.memset(ones_row, 1.0)

            # PE warm-up: ~5us of dummy fp32 matmul keeps the HAM activity
            # window busy while the first x tiles stream in, so the real
            # matmuls start at the 2.4 GHz p-state. The tiny DRAM store
            # keeps the chain live (overwritten by the real outT store).
            # (ones x ones = all-ones, so writing one element back into
            # ones_row is a no-op value-wise but keeps the chain live via
            # ones_row's later real consumers)
            wp = None
            for i in range(8):
                wp = ptr_pool.tile([P, P], F32, tag="tr")
                nc.tensor.matmul(wp, lhsT=ones_row, rhs=ones_row, start=True, stop=True)
            nc.vector.tensor_copy(out=ones_row[0:1, 0:1], in_=wp[0:1, 0:1])

            # weights/biases on the (otherwise idle) SWDGE queue so the sync
            # queue starts streaming x immediately; stage fp32 from DRAM,
            # round to f32r via DVE copy (PE f32r operands must come from a
            # rounding producer).
            w_st = const.tile([P, 3 * NH, D], F32, tag="w_st")
            for wi, w_h in enumerate((wq, wk, wv)):
                for hb in range(NH):
                    nc.gpsimd.dma_start(
                        out=w_st[:, wi * NH + hb, :], in_=w_h[hb * P:(hb + 1) * P, :]
                    )
            w_sb = const.tile([P, 3 * NH, D], F32R, tag="w_sb")
            nc.vector.tensor_copy(out=w_sb, in_=w_st)

            b_sb = const.tile([D, 3], F32, tag="b_sb")
            for bi, b_h in enumerate((bq, bk, bv)):
                nc.gpsimd.dma_start(
                    out=b_sb[:, bi:bi + 1],
                    in_=b_h[:].rearrange("(d one) -> d one", one=1),
                )

            # ---- persistent activations ----
            QT = persist.tile([D, S], F32R, tag="QT")
            KT = persist.tile([D, S], F32R, tag="KT")
            VTp = persist.tile([D + 1, S], F32, tag="VTp")   # row 64 = ones
            Vp = persist.tile([P, NT, D + 1], F32R, tag="Vp")
            outT_sb = persist.tile([D, S], F32, tag="outT")

            nc.vector.memset(VTp[D:D + 1, :], 1.0)

            def proj(wi, dst, xt, sblk):
                acc = pmm_pool.tile([P, 2, SB], F32, tag="mm")
                for hb in range(NH):
                    nc.tensor.matmul(
                        acc[0:D, 0, :], lhsT=w_sb[:, wi * NH + hb, :],
                        rhs=xt[:, hb, :],
                        start=(hb == 0), stop=(hb == NH - 1),
                    )
                nc.scalar.activation(
                    out=dst[0:D, sblk * SB:(sblk + 1) * SB],
                    in_=acc[0:D, 0, :], func=IDENT_FN,
                    bias=b_sb[:, wi:wi + 1], scale=1.0,
                )

            # ---- phase 1a: xT tiles + K projections (K first: phase 2's
            # scores matmuls depend on the full KT) ----
            xts = []
            for sblk in range(NSB):
                xt = xt_pool.tile([P, NH, SB], F32R, tag="xt")
                for st in range(4):
                    row0 = (sblk * 4 + st) * P
                    xn = xn_pool.tile([P, H], F32, tag="xn")
                    dma_eng = nc.sync if st % 2 == 0 else nc.scalar
                    dma_eng.dma_start(out=xn, in_=x[row0:row0 + P, :])
                    for hb in range(NH):
                        ps = ptr_pool.tile([P, P], F32, tag="tr")
                        nc.tensor.transpose(ps, xn[:, hb * P:(hb + 1) * P], ident)
                        dst_ap = xt[:, hb, st * P:(st + 1) * P]
                        # split PSUM evacuation across DVE and ACT
                        if hb % 2 == 0:
                            nc.vector.tensor_copy(out=dst_ap, in_=ps)
                        else:
                            nc.scalar.activation(out=dst_ap, in_=ps, func=COPY_FN)
                proj(1, KT, xt, sblk)
                xts.append(xt)

            # ---- phase 1b: Q/V projections + V' tiles ----
            for sblk in range(NSB):
                xt = xts[sblk]
                proj(0, QT, xt, sblk)
                proj(2, VTp, xt, sblk)
                for tb in range(sblk * 4, sblk * 4 + 4):
                    ps = ptr_pool.tile([P, D + 1], F32, tag="tr")
                    nc.tensor.transpose(
                        ps, VTp[:, tb * P:(tb + 1) * P], ident[0:D + 1, 0:D + 1]
                    )
                    nc.vector.tensor_copy(out=Vp[:, tb, :], in_=ps)

            # ---- phase 2: scoresT -> exp -> EV (+rowsum) -> normalize ----
            # passes over s-block groups, t-block outer within a pass so the
            # KT weight load amortizes over the group's scores matmuls and Vp
            # over its EV matmuls (exposed LDWEIGHTS is the phase-2 PE tax).
            # sc is one 2-bank tile per t-block so the scores matmuls stay
            # adjacent in the PE stream. The last passes are single-width so
            # the final normalize+store tail stays short.
            for sblks in ((0, 1), (2,), (3,)):
                W = len(sblks)
                c0 = sblks[0] * SB
                cols = slice(c0, c0 + W * SB)
                o2 = po_pool.tile([D + 1, 2, SB], F32, tag="o")
                ets = []
                for tb in range(NT):
                    et2 = et_pool.tile([P, W, SB], F32R, tag="et")
                    sc = pmm_pool.tile([P, 2, SB], F32, tag="mm")
                    for h in range(W):
                        nc.tensor.matmul(
                            sc[:, h, :],
                            lhsT=KT[:, tb * P:(tb + 1) * P],
                            rhs=QT[:, c0 + h * SB:c0 + (h + 1) * SB],
                            start=True, stop=True,
                            skip_group_check=True,
                        )
                    nc.scalar.activation(
                        out=et2[:, :, :].rearrange("p h s -> p (h s)"),
                        in_=sc[:, 0:W, :].rearrange("p h s -> p (h s)"),
                        func=EXP, scale=0.125,
                    )
                    for h in range(W):
                        nc.tensor.matmul(
                            o2[:, h, :], lhsT=Vp[:, tb, :], rhs=et2[:, h, :],
                            start=(tb == 0), stop=(tb == NT - 1),
                            skip_group_check=True,
                        )
                    ets.append(et2)
                # rowsum (row 64 of o2) -> broadcast -> reciprocal
                rsum = rb_pool.tile([1, W, SB], F32, tag="rs")
                nc.vector.tensor_copy(out=rsum, in_=o2[D:D + 1, 0:W, :])
                rb2 = rb_pool.tile([P, W, SB], F32, tag="rb")
                for h in range(W):
                    bc = ptr_pool.tile([P, SB], F32, tag="tr")
                    nc.tensor.matmul(
                        bc, lhsT=ones_row, rhs=rsum[0:1, h, :],
                        start=True, stop=True,
                    )
                    nc.vector.reciprocal_approx_fast(out=rb2[:, h, :], in_=bc)
                nc.vector.tensor_mul(
                    outT_sb[:, cols].rearrange("d (h s) -> d h s", h=W),
                    o2[0:D, 0:W, :], rb2[0:D, :, :],
                )
                rb_flat = rb2[:, :, :].rearrange("p h s -> p (h s)")
                for tb in range(NT):
                    pr = pr_pool.tile([P, W * SB], F32, tag="pr")
                    nc.vector.tensor_mul(
                        pr,
                        ets[tb][:, :, :].bitcast(F32).rearrange("p h s -> p (h s)"),
                        rb_flat,
                    )
                    nc.sync.dma_start(
                        out=probsT[tb * P:(tb + 1) * P, cols], in_=pr
                    )

            nc.sync.dma_start(out=outT[:, :], in_=outT_sb[:, :])

    nc.compile()
    return nc


def _get_compiled():
    global _COMPILED
    if _COMPILED is None:
        _ensure_axon_hooks()
        _COMPILED = _build()
    return _COMPILED


def kernel(x, Wq, bq, Wk, bk, Wv, bv, trace=False, trace_kwargs=None):
    from concourse.bass_utils import run_bass_kernel_spmd

    nc = _get_compiled()

    x = np.ascontiguousarray(np.asarray(x, dtype=np.float32))
    weights = {
        "Wq": np.ascontiguousarray(np.asarray(Wq, dtype=np.float32)),
        "bq": np.ascontiguousarray(np.asarray(bq, dtype=np.float32)),
        "Wk": np.ascontiguousarray(np.asarray(Wk, dtype=np.float32)),
        "bk": np.ascontiguousarray(np.asarray(bk, dtype=np.float32)),
        "Wv": np.ascontiguousarray(np.asarray(Wv, dtype=np.float32)),
        "bv": np.ascontiguousarray(np.asarray(bv, dtype=np.float32)),
    }
    in_maps = [{"x": x[b], **weights} for b in range(B)]

    kw = dict(trace_kwargs or {})
    res = run_bass_kernel_spmd(nc, in_maps, list(range(N_CORES)), trace=trace, **kw)

    out = np.empty((B, S, D), dtype=np.float32)
    probs = np.empty((B, S, S), dtype=np.float32)
    for b in range(B):
        out[b] = res.results[b]["outT"].T
        probs[b] = res.results[b]["probsT"].T

    if trace:
        kernel.last_result = res
    return (out, probs)


# revision 52
# speedup vs baseline: 1.1885x; 1.1885x over previous
"""Single-head attention (Q/K/V proj + softmax + PV) on 8 TRN2 NeuronCores.

Sharding: data-parallel over batch (B=8 -> 1 batch element per core);
Q/K/V weights replicated. Each core computes, for its batch b:
  probsT_b = softmax(x_b Wq+bq @ (x_b Wk+bk)^T / 8, axis=t)^T   [t, s] layout
  outT_b   = (probs_b @ (x_b Wv+bv))^T                          [d, s] layout
The transposed layouts keep every on-chip matmul in the natural
TensorE form (contract dim on partitions) with no large on-chip
transpose of the S x S matrix; the host returns transposed views.

Device-side math (per core):
  xT (PE-transpose of x tiles) -> QT/KT/VT = W^T xT (+bias)
  scoresT[t,s] = K Q^T  (lhsT = KT tile, rhs = QT block)
  ET = exp(0.125 * scoresT)            (ScalarE, fused scale)
  [outT; rowsum] = [V | 1]^T ET        (ones column folded into V)
  probsT = ET * (1/rowsum) broadcast   (PE ones-broadcast + DVE recip)

Matmul operands use float32r (single-pass PE mode, ~1.6e-4 matmul rel
err vs 4-cycle/row full fp32) -- the kernel is memory-bound and fp32
PE would otherwise dominate. Phase 1 computes all K projections first
so phase 2's scores matmuls can overlap the remaining Q/V projections.
"""

import sys
import types

import numpy as np

B, S, H, D = 8, 2048, 768, 64
P = 128          # partitions
SB = 512         # s-block width (PSUM bank)
NSB = S // SB    # 4 s-blocks
NT = S // P      # 16 t-blocks
NH = H // P      # 6 h-blocks
N_CORES = 8

_COMPILED = None


def _ensure_axon_hooks():
    """bass_utils imports antenv.axon_hooks under axon when trace=True;
    provide the real ctypes hook (or a stub) if the image lacks it."""
    if "antenv.axon_hooks" in sys.modules:
        return
    try:
        import antenv.axon_hooks  # noqa: F401
        return
    except ImportError:
        pass
    hook = None
    try:
        from trn_agent_boot.trn_boot import _ntff_profile_via_ctypes
        hook = _ntff_profile_via_ctypes("/opt/axon/libaxon_pjrt.so")
    except Exception:
        hook = None
    mod = types.ModuleType("antenv.axon_hooks")
    mod.get_axon_ntff_profile_hook = lambda: hook
    mod.set_axon_ntff_profile_hook = lambda h: None
    sys.modules["antenv.axon_hooks"] = mod


def _build():
    import concourse.bacc as bacc
    import concourse.tile as tile
    import concourse.mybir as mybir
    from concourse.masks import make_identity

    F32 = mybir.dt.float32
    F32R = mybir.dt.float32r
    EXP = mybir.ActivationFunctionType.Exp
    IDENT_FN = mybir.ActivationFunctionType.Identity
    COPY_FN = mybir.ActivationFunctionType.Copy

    nc = bacc.Bacc(None, target_bir_lowering=False, debug=False)

    x = nc.dram_tensor("x", [S, H], F32, kind="ExternalInput")
    wq = nc.dram_tensor("Wq", [H, D], F32, kind="ExternalInput")
    bq = nc.dram_tensor("bq", [D], F32, kind="ExternalInput")
    wk = nc.dram_tensor("Wk", [H, D], F32, kind="ExternalInput")
    bk = nc.dram_tensor("bk", [D], F32, kind="ExternalInput")
    wv = nc.dram_tensor("Wv", [H, D], F32, kind="ExternalInput")
    bv = nc.dram_tensor("bv", [D], F32, kind="ExternalInput")
    probsT = nc.dram_tensor("probsT", [S, S], F32, kind="ExternalOutput")
    outT = nc.dram_tensor("outT", [D, S], F32, kind="ExternalOutput")

    with tile.TileContext(nc) as tc:
        with (
            tc.tile_pool(name="const", bufs=1) as const,
            tc.tile_pool(name="persist", bufs=1) as persist,
            tc.tile_pool(name="xn", bufs=3) as xn_pool,
            tc.tile_pool(name="xt", bufs=4) as xt_pool,
            tc.tile_pool(name="et", bufs=17) as et_pool,
            tc.tile_pool(name="rb", bufs=2) as rb_pool,
            tc.tile_pool(name="pr", bufs=5) as pr_pool,
            tc.tile_pool(name="ptr", bufs=2, space="PSUM") as ptr_pool,
            tc.tile_pool(name="pmm", bufs=2, space="PSUM") as pmm_pool,
            tc.tile_pool(name="po", bufs=1, space="PSUM") as po_pool,
        ):
            # ---- constants ----
            ident = const.tile([P, P], F32)
            make_identity(nc, ident)
            ones_row = const.tile([1, P], F32)
            nc.vector.memset(ones_row, 1.0)

            # weights/biases on the (otherwise idle) SWDGE queue so the sync
            # queue starts streaming x immediately; stage fp32 from DRAM,
            # round to f32r via DVE copy (PE f32r operands must come from a
            # rounding producer).
            w_st = const.tile([P, 3 * NH, D], F32, tag="w_st")
            for wi, w_h in enumerate((wq, wk, wv)):
                for hb in range(NH):
                    nc.gpsimd.dma_start(
                        out=w_st[:, wi * NH + hb, :], in_=w_h[hb * P:(hb + 1) * P, :]
                    )
            w_sb = const.tile([P, 3 * NH, D], F32R, tag="w_sb")
            nc.vector.tensor_copy(out=w_sb, in_=w_st)

            b_sb = const.tile([D, 3], F32, tag="b_sb")
            for bi, b_h in enumerate((bq, bk, bv)):
                nc.gpsimd.dma_start(
                    out=b_sb[:, bi:bi + 1],
                    in_=b_h[:].rearrange("(d one) -> d one", one=1),
                )

            # ---- persistent activations ----
            QT = persist.tile([D, S], F32R, tag="QT")
            KT = persist.tile([D, S], F32R, tag="KT")
            VTp = persist.tile([D + 1, S], F32, tag="VTp")   # row 64 = ones
            Vp = persist.tile([P, NT, D + 1], F32R, tag="Vp")
            outT_sb = persist.tile([D, S], F32, tag="outT")

            nc.vector.memset(VTp[D:D + 1, :], 1.0)

            def proj(wi, dst, xt, sblk):
                acc = pmm_pool.tile([P, 2, SB], F32, tag="mm")
                for hb in range(NH):
                    nc.tensor.matmul(
                        acc[0:D, 0, :], lhsT=w_sb[:, wi * NH + hb, :],
                        rhs=xt[:, hb, :],
                        start=(hb == 0), stop=(hb == NH - 1),
                    )
                nc.scalar.activation(
                    out=dst[0:D, sblk * SB:(sblk + 1) * SB],
                    in_=acc[0:D, 0, :], func=IDENT_FN,
                    bias=b_sb[:, wi:wi + 1], scale=1.0,
                )

            # ---- phase 1a: xT tiles + K projections (K first: phase 2's
            # scores matmuls depend on the full KT) ----
            xts = []
            for sblk in range(NSB):
                xt = xt_pool.tile([P, NH, SB], F32R, tag="xt")
                for st in range(4):
                    row0 = (sblk * 4 + st) * P
                    xn = xn_pool.tile([P, H], F32, tag="xn")
                    dma_eng = nc.sync if st % 2 == 0 else nc.scalar
                    dma_eng.dma_start(out=xn, in_=x[row0:row0 + P, :])
                    for hb in range(NH):
                        ps = ptr_pool.tile([P, P], F32, tag="tr")
                        nc.tensor.transpose(ps, xn[:, hb * P:(hb + 1) * P], ident)
                        dst_ap = xt[:, hb, st * P:(st + 1) * P]
                        # split PSUM evacuation across DVE and ACT
                        if hb % 2 == 0:
                            nc.vector.tensor_copy(out=dst_ap, in_=ps)
                        else:
                            nc.scalar.activation(out=dst_ap, in_=ps, func=COPY_FN)
                proj(1, KT, xt, sblk)
                xts.append(xt)

            # ---- phase 1b: Q/V projections + V' tiles ----
            for sblk in range(NSB):
                xt = xts[sblk]
                proj(0, QT, xt, sblk)
                proj(2, VTp, xt, sblk)
                for tb in range(sblk * 4, sblk * 4 + 4):
                    ps = ptr_pool.tile([P, D + 1], F32, tag="tr")
                    nc.tensor.transpose(
                        ps, VTp[:, tb * P:(tb + 1) * P], ident[0:D + 1, 0:D + 1]
                    )
                    nc.vector.tensor_copy(out=Vp[:, tb, :], in_=ps)

            # ---- phase 2: scoresT -> exp -> EV (+rowsum) -> normalize ----
            # passes over s-block groups, t-block outer within a pass so the
            # KT weight load amortizes over the group's scores matmuls and Vp
            # over its EV matmuls (exposed LDWEIGHTS is the phase-2 PE tax).
            # sc is one 2-bank tile per t-block so the scores matmuls stay
            # adjacent in the PE stream. The last passes are single-width so
            # the final normalize+store tail stays short.
            for sblks in ((0, 1), (2,), (3,)):
                W = len(sblks)
                c0 = sblks[0] * SB
                cols = slice(c0, c0 + W * SB)
                o2 = po_pool.tile([D + 1, 2, SB], F32, tag="o")
                ets = []
                for tb in range(NT):
                    et2 = et_pool.tile([P, W, SB], F32R, tag="et")
                    sc = pmm_pool.tile([P, 2, SB], F32, tag="mm")
                    for h in range(W):
                        nc.tensor.matmul(
                            sc[:, h, :],
                            lhsT=KT[:, tb * P:(tb + 1) * P],
                            rhs=QT[:, c0 + h * SB:c0 + (h + 1) * SB],
                            start=True, stop=True,
                            skip_group_check=True,
                        )
                    nc.scalar.activation(
                        out=et2[:, :, :].rearrange("p h s -> p (h s)"),
                        in_=sc[:, 0:W, :].rearrange("p h s -> p (h s)"),
                        func=EXP, scale=0.125,
                    )
                    for h in range(W):
                        nc.tensor.matmul(
                            o2[:, h, :], lhsT=Vp[:, tb, :], rhs=et2[:, h, :],
                            start=(tb == 0), stop=(tb == NT - 1),
                            skip_group_check=True,
                        )
                    ets.append(et2)
                # rowsum (row 64 of o2) -> broadcast -> reciprocal
                rsum = rb_pool.tile([1, W, SB], F32, tag="rs")
                nc.vector.tensor_copy(out=rsum, in_=o2[D:D + 1, 0:W, :])
                rb2 = rb_pool.tile([P, W, SB], F32, tag="rb")
                for h in range(W):
                    bc = ptr_pool.tile([P, SB], F32, tag="tr")
                    nc.tensor.matmul(
                        bc, lhsT=ones_row, rhs=rsum[0:1, h, :],
                        start=True, stop=True,
                    )
                    nc.vector.reciprocal_approx_fast(out=rb2[:, h, :], in_=bc)
                nc.vector.tensor_mul(
                    outT_sb[:, cols].rearrange("d (h s) -> d h s", h=W),
                    o2[0:D, 0:W, :], rb2[0:D, :, :],
                )
                rb_flat = rb2[:, :, :].rearrange("p h s -> p (h s)")
                for tb in range(NT):
                    pr = pr_pool.tile([P, W * SB], F32, tag="pr")
                    nc.vector.tensor_mul(
                        pr,
                        ets[tb][:, :, :].bitcast(F32).rearrange("p h s -> p (h s)"),
                        rb_flat,
                    )
                    nc.sync.dma_start(
                        out=probsT[tb * P:(tb + 1) * P, cols], in_=pr
                    )

            nc.sync.dma_start(out=outT[:, :], in_=outT_sb[:, :])

    nc.compile()
    return nc


def _get_compiled():
    global _COMPILED
    if _COMPILED is None:
        _ensure_axon_hooks()
        _COMPILED = _build()
    return _COMPILED


def kernel(x, Wq, bq, Wk, bk, Wv, bv, trace=False, trace_kwargs=None):
    from concourse.bass_utils import run_bass_kernel_spmd

    nc = _get_compiled()

    x = np.ascontiguousarray(np.asarray(x, dtype=np.float32))
    weights = {
        "Wq": np.ascontiguousarray(np.asarray(Wq, dtype=np.float32)),
        "bq": np.ascontiguousarray(np.asarray(bq, dtype=np.float32)),
        "Wk": np.ascontiguousarray(np.asarray(Wk, dtype=np.float32)),
        "bk": np.ascontiguousarray(np.asarray(bk, dtype=np.float32)),
        "Wv": np.ascontiguousarray(np.asarray(Wv, dtype=np.float32)),
        "bv": np.ascontiguousarray(np.asarray(bv, dtype=np.float32)),
    }
    in_maps = [{"x": x[b], **weights} for b in range(B)]

    kw = dict(trace_kwargs or {})
    res = run_bass_kernel_spmd(nc, in_maps, list(range(N_CORES)), trace=trace, **kw)

    out = np.empty((B, S, D), dtype=np.float32)
    probs = np.empty((B, S, S), dtype=np.float32)
    for b in range(B):
        out[b] = res.results[b]["outT"].T
        probs[b] = res.results[b]["probsT"].T

    if trace:
        kernel.last_result = res
    return (out, probs)
